# revision 12
# baseline (speedup 1.0000x reference)
"""Local (windowed) attention scores kernel for Trainium2, 8 NeuronCores.

Computes softmax(Q_win @ [K_prev|K_self|K_next]^T / sqrt(d)) per 128-wide
window, drops windows 2 and 34, zeros the padded edge regions of windows 0
and 63.  Data-parallel over the collapsed batch*heads axis (32 -> 4 per core).

Design (v3):
  * All device math in fp16 (PE 1 cyc/row, DVE 2x/4x packed modes, 8x the
    mantissa of bf16).  GPSIMD casts the fp32 inputs.
  * Inputs loaded with fully contiguous HBM reads (16KB/partition; partition
    p holds tokens [64p, 64p+64)).  K^T is kept "a-major" (column (a,p) =
    token 64p+a) and the score matmuls use a strided moving AP over it, so
    output columns come out in a fixed permutation undone on the host.
    Q^T is stored token-major (strided DVE copy) because the stationary
    operand must have a single free dim.
  * ACT does exp straight out of score-PSUM into an fp16 stage (2 windows
    per instruction), plus the K^T PSUM->SBUF copies and 1-in-3 group
    normalizes (load balance with DVE).
  * Softmax denominators come from a pairwise tensor_tensor fold tree on
    DVE (2x packed mode) + one segmented 1x tail reduce -- the per-window
    accum-reduce op only has a 1x uop and was the previous bottleneck.
  * Output written to HBM in fp16 (halves the dominant DMA stream) and
    upcast on the host.

Scheduling constraint: walrus places every sync wait of a Matmult on the
LDWEIGHTS struct, which has a single wait slot -- each PE instruction may
wait on at most ONE semaphore.  Tiny "absorber" matmuls soak the
Pool(cast)/DVE(q-copies)/ACT(k-copies) ticks so every real PE instruction
carries at most one cross-engine wait.
"""

import sys

for _p in ("/opt/trn_rl_repo", "/opt/trn_rl_repo/concourse"):
    if _p not in sys.path:
        sys.path.insert(0, _p)

import numpy as np

B, H, N, D = 4, 8, 8192, 64
BH = B * H                      # 32
NCORES = 8
BHC = BH // NCORES              # 4 batch-heads per core
W = 128                         # window size
NW = N // W                     # 64 windows
EXCLUDED = (2, 34)
REMAINING = [i for i in range(NW) if i not in EXCLUDED]
NOUT = len(REMAINING)           # 62
J = 3 * W                       # 384 keys per query window
SCALE = float(D) ** -0.5        # 0.125

GS = 6                          # output windows per stage buffer / out-DMA
TA = 8                          # transpose slots per PSUM tile (1 bank fp16)

_cached_nc = None


def _build():
    import concourse.bass as bass
    import concourse.mybir as mybir
    import concourse.tile as tile
    from concourse import bacc
    from concourse.masks import make_identity
    from concourse.tile import add_dep_helper

    fp32 = mybir.dt.float32
    fp16 = mybir.dt.float16
    mult = mybir.AluOpType.mult
    add = mybir.AluOpType.add

    nc = bacc.Bacc("TRN2", target_bir_lowering=False, debug=False)
    q = nc.dram_tensor("q", [BHC, N, D], fp32, kind="ExternalInput").ap()
    k = nc.dram_tensor("k", [BHC, N, D], fp32, kind="ExternalInput").ap()
    out = nc.dram_tensor("out", [BHC, NOUT, W, J], fp16, kind="ExternalOutput").ap()

    def raw(inst):
        return inst.ins if hasattr(inst, "ins") and not isinstance(inst.ins, list) else inst

    with tile.TileContext(nc) as tc:
        from contextlib import ExitStack

        with ExitStack() as ctx:
            singles = ctx.enter_context(tc.tile_pool(name="singles", bufs=1))
            kin_pool = ctx.enter_context(tc.tile_pool(name="kin", bufs=2))
            qin_pool = ctx.enter_context(tc.tile_pool(name="qin", bufs=2))
            kbf_pool = ctx.enter_context(tc.tile_pool(name="kbf", bufs=2))
            qbf_pool = ctx.enter_context(tc.tile_pool(name="qbf", bufs=2))
            kt_pool = ctx.enter_context(tc.tile_pool(name="kt", bufs=2))
            qt_pool = ctx.enter_context(tc.tile_pool(name="qt", bufs=2))
            sa_pool = ctx.enter_context(tc.tile_pool(name="stageA", bufs=3))
            sb_pool = ctx.enter_context(tc.tile_pool(name="stageB", bufs=2))
            sums_pool = ctx.enter_context(tc.tile_pool(name="sums", bufs=2))
            tpsum = ctx.enter_context(tc.tile_pool(name="tpsum", bufs=1, space="PSUM"))
            spsum = ctx.enter_context(tc.tile_pool(name="spsum", bufs=2, space="PSUM"))
            scrapp = ctx.enter_context(tc.tile_pool(name="scrap", bufs=1, space="PSUM"))

            identh = singles.tile([128, 128], fp16)
            make_identity(nc, identh)
            scrap = scrapp.tile([2, 2], fp32, tag="scrap")
            # absorb the gpsimd (ident) wait into PE's clock once
            nc.tensor.matmul(scrap, identh[:, :2], identh[:, :2], start=True, stop=True)

            def absorber(lhs2, rhs2, dep=None, why="absorber"):
                """1-wait PE matmul absorbing a cross-engine dependency."""
                mm = nc.tensor.matmul(scrap, lhs2, rhs2, start=True, stop=True)
                if dep is not None:
                    add_dep_helper(raw(mm), raw(dep), False, why)
                return mm

            group_rr = 0  # group counter for the normalize engine rotation
            for bh in range(BHC):
                # ---- contiguous loads: partition p <- tokens [64p, 64p+64) ----
                ktile = kin_pool.tile([128, 64, D], fp32, tag="kin")
                qtile = qin_pool.tile([128, 64, D], fp32, tag="qin")
                nc.sync.dma_start(out=ktile, in_=k[bh].rearrange("(p a) d -> p a d", p=128))
                nc.sync.dma_start(out=qtile, in_=q[bh].rearrange("(p a) d -> p a d", p=128))

                kbf = kbf_pool.tile([128, 64, D], fp16, tag="kbf")
                qbf = qbf_pool.tile([128, 64, D], fp16, tag="qbf")
                kt = kt_pool.tile([D, 64, 128], fp16, tag="kt")
                qt = qt_pool.tile([D, N], fp16, tag="qt")
                qt_pa = qt.rearrange("d (p a) -> d p a", p=128)

                # casts chunked in halves; k/q transpose tiles interleaved so
                # the ACT (k) and DVE (q) copy streams both start early
                for h in range(2):
                    hs = slice(32 * h, 32 * h + 32)
                    nc.gpsimd.tensor_copy(out=kbf[:, hs], in_=ktile[:, hs])
                    nc.gpsimd.tensor_copy(out=qbf[:, hs], in_=qtile[:, hs])
                    ab_k = absorber(kbf[:, 32 * h, :2], identh[:, :2], dep=None)
                    ab_q = absorber(qbf[:, 32 * h, :2], identh[:, :2], dep=None)
                    first_k = first_q = True
                    for a0 in range(32 * h, 32 * h + 32, TA):
                        # K tile: a-major contiguous both sides -> DVE 2x copy
                        tpk = tpsum.tile([D, TA, 128], fp16, tag="t")
                        for t in range(TA):
                            mm = nc.tensor.transpose(tpk[:, t, :], kbf[:, a0 + t, :], identh)
                            if first_k:
                                add_dep_helper(raw(mm), raw(ab_k), False, "k after absorber")
                                first_k = False
                        nc.vector.tensor_copy(out=kt[:, a0 : a0 + TA, :], in_=tpk)
                        # Q tile: token-major dst is strided -> ACT copy
                        tpq = tpsum.tile([D, TA, 128], fp16, tag="t2")
                        for t in range(TA):
                            mm = nc.tensor.transpose(tpq[:, t, :], qbf[:, a0 + t, :], identh)
                            if first_q:
                                add_dep_helper(raw(mm), raw(ab_q), False, "q after absorber")
                                first_q = False
                        nc.scalar.copy(
                            out=qt_pa[:, :, a0 : a0 + TA],
                            in_=tpq.rearrange("d t p -> d p t"),
                        )

                # absorbers soaking the DVE (qt) and ACT (kt) copy ticks so the
                # score matmuls' only cross-engine wait is the ACT psum-recycle
                absorber(kt[:, 0, :2], identh[:64, :2], dep=None)
                absorber(qt[:64, :2], identh[:64, :2], dep=None)

                # ---- per output-window group ----
                o0 = 0
                while o0 < NOUT:
                    gs = min(GS, NOUT - o0)
                    stage_a = sa_pool.tile([128, GS, J], fp16, tag="sa")
                    stage_b = sb_pool.tile([128, GS, J], fp16, tag="sb")
                    sums = sums_pool.tile([128, GS], fp32, tag="sums")
                    recip = sums_pool.tile([128, GS], fp32, tag="recip")
                    for p0 in range(0, gs, 2):
                        sc = spsum.tile([128, 2, 512], fp32, tag="s")
                        plens = []
                        for s2 in range(2):
                            s = p0 + s2
                            wi = REMAINING[o0 + s]
                            lo = max(0, 2 * wi - 2)
                            hi = min(128, 2 * wi + 4)
                            cols = 64 * (hi - lo)
                            plens.append(cols)
                            nc.tensor.matmul(
                                sc[:, s2, :cols],
                                qt[:, wi * W : (wi + 1) * W],
                                kt[:, :, lo:hi],
                                start=True,
                                stop=True,
                            )
                        # exp on ACT straight out of PSUM into the fp16 stage
                        if plens[0] == plens[1] == J:
                            nc.scalar.activation(
                                stage_a[:, p0 : p0 + 2, :],
                                sc[:, :, :J],
                                mybir.ActivationFunctionType.Exp,
                                scale=SCALE,
                            )
                        else:
                            for s2 in range(2):
                                nc.scalar.activation(
                                    stage_a[:, p0 + s2, : plens[s2]],
                                    sc[:, s2, : plens[s2]],
                                    mybir.ActivationFunctionType.Exp,
                                    scale=SCALE,
                                )
                                if plens[s2] < J:
                                    # zero the tail so the fold sums stay exact
                                    nc.vector.memset(stage_a[:, p0 + s2, plens[s2] :], 0.0)
                        # per-window flat L1 fold (2x packed): B[s,0:192] = A+A
                        for s2 in range(2):
                            s = p0 + s2
                            nc.vector.tensor_tensor(
                                out=stage_b[:, s, 0:192], in0=stage_a[:, s, 0:192],
                                in1=stage_a[:, s, 192:384], op=add)

                    # segmented tail reduce (1x) + reciprocal
                    nc.vector.tensor_reduce(
                        out=sums[:, :gs], in_=stage_b[:, :gs, 0:192],
                        axis=mybir.AxisListType.X, op=add)
                    nc.vector.reciprocal(recip[:, :gs], sums[:, :gs])

                    # ---- normalize A -> B, spread over DVE / ACT / Pool ----
                    g4 = group_rr % 4
                    for s in range(gs):
                        eng = "dve"
                        if s == 0:
                            eng = "pool"
                        elif s == 3:
                            eng = "act"
                        elif s == 1 and g4 == 1:
                            eng = "act"
                        elif s == 1 and g4 == 3:
                            eng = "pool"
                        if eng == "act":
                            nc.scalar.mul(
                                stage_b[:, s, :], stage_a[:, s, :], recip[:, s : s + 1])
                        elif eng == "pool":
                            nc.gpsimd.tensor_scalar(
                                out=stage_b[:, s, :], in0=stage_a[:, s, :],
                                scalar1=recip[:, s : s + 1], scalar2=None, op0=mult)
                        else:
                            nc.vector.tensor_scalar(
                                out=stage_b[:, s, :], in0=stage_a[:, s, :],
                                scalar1=recip[:, s : s + 1], scalar2=None, op0=mult)
                    group_rr += 1
                    dst = out[bh, o0 : o0 + gs].rearrange("w c j -> c w j")
                    nc.sync.dma_start(out=dst, in_=stage_b[:, :gs, :])
                    o0 += gs
    nc.compile()
    return nc


# ---- host-side permutation maps -------------------------------------------
# Output rows are already in query order.  Stage col a*6+dp holds key token
# 64*(2(w-1)+dp)+a, i.e. j_ref = 64*dp+a -> col(j) = (j%64)*6 + j//64.
# Window 0 (4 p-slots, j_ref>=128): col = ((j-128)%64)*4 + (j-128)//64.
# Window 63 (4 p-slots, j_ref<256): col = (j%64)*4 + j//64.
_JM = ((np.arange(J) % 64) * 6 + np.arange(J) // 64).astype(np.intp)
_J0 = (((np.arange(128, J) - 128) % 64) * 4 + (np.arange(128, J) - 128) // 64).astype(np.intp)
_J63 = ((np.arange(256) % 64) * 4 + np.arange(256) // 64).astype(np.intp)


def _assemble(raw):
    """raw: [BH, NOUT, 128, 384] fp16 device layout -> fp32 reference layout."""
    res = np.empty((BH, NOUT, W, J), np.float32)
    res[:, 1 : NOUT - 1] = raw[:, 1 : NOUT - 1][..., _JM]
    res[:, 0, :, :128] = 0.0
    res[:, 0, :, 128:] = raw[:, 0][..., _J0]
    res[:, NOUT - 1, :, :256] = raw[:, NOUT - 1][..., _J63]
    res[:, NOUT - 1, :, 256:] = 0.0
    return res


def _run(q, k, trace=False):
    from concourse.bass_utils import run_bass_kernel_spmd

    global _cached_nc
    if _cached_nc is None:
        _cached_nc = _build()
    nc = _cached_nc

    q = np.ascontiguousarray(np.asarray(q), dtype=np.float32).reshape(BH, N, D)
    k = np.ascontiguousarray(np.asarray(k), dtype=np.float32).reshape(BH, N, D)
    in_maps = [
        {
            "q": np.ascontiguousarray(q[c * BHC : (c + 1) * BHC]),
            "k": np.ascontiguousarray(k[c * BHC : (c + 1) * BHC]),
        }
        for c in range(NCORES)
    ]
    res = run_bass_kernel_spmd(nc, in_maps, core_ids=list(range(NCORES)), trace=trace)
    raw = np.concatenate([np.asarray(res.results[c]["out"]) for c in range(NCORES)], axis=0)
    return _assemble(raw), res


def kernel(q, k):
    out, _ = _run(q, k, trace=False)
    return out


# revision 13
# speedup vs baseline: 1.7998x; 1.7998x over previous
"""Local (windowed) attention scores kernel for Trainium2, 8 NeuronCores.

Computes softmax(Q_win @ [K_prev|K_self|K_next]^T / sqrt(d)) per 128-wide
window, drops windows 2 and 34, zeros the padded edge regions of windows 0
and 63.  Data-parallel over the collapsed batch*heads axis (32 -> 4 per core).

Design (v3):
  * All device math in fp16 (PE 1 cyc/row, DVE 2x/4x packed modes, 8x the
    mantissa of bf16).  GPSIMD casts the fp32 inputs.
  * Inputs loaded with fully contiguous HBM reads (16KB/partition; partition
    p holds tokens [64p, 64p+64)).  K^T is kept "a-major" (column (a,p) =
    token 64p+a) and the score matmuls use a strided moving AP over it, so
    output columns come out in a fixed permutation undone on the host.
    Q^T is stored token-major (strided DVE copy) because the stationary
    operand must have a single free dim.
  * ACT does exp straight out of score-PSUM into an fp16 stage (2 windows
    per instruction), plus the K^T PSUM->SBUF copies and 1-in-3 group
    normalizes (load balance with DVE).
  * Softmax denominators come from a pairwise tensor_tensor fold tree on
    DVE (2x packed mode) + one segmented 1x tail reduce -- the per-window
    accum-reduce op only has a 1x uop and was the previous bottleneck.
  * Output written to HBM in fp16 (halves the dominant DMA stream) and
    upcast on the host.

Scheduling constraint: walrus places every sync wait of a Matmult on the
LDWEIGHTS struct, which has a single wait slot -- each PE instruction may
wait on at most ONE semaphore.  Tiny "absorber" matmuls soak the
Pool(cast)/DVE(q-copies)/ACT(k-copies) ticks so every real PE instruction
carries at most one cross-engine wait.
"""

import sys

for _p in ("/opt/trn_rl_repo", "/opt/trn_rl_repo/concourse"):
    if _p not in sys.path:
        sys.path.insert(0, _p)

import numpy as np

B, H, N, D = 4, 8, 8192, 64
BH = B * H                      # 32
NCORES = 8
BHC = BH // NCORES              # 4 batch-heads per core
W = 128                         # window size
NW = N // W                     # 64 windows
EXCLUDED = (2, 34)
REMAINING = [i for i in range(NW) if i not in EXCLUDED]
NOUT = len(REMAINING)           # 62
J = 3 * W                       # 384 keys per query window
SCALE = float(D) ** -0.5        # 0.125

GS = 6                          # output windows per stage buffer / out-DMA
TA = 8                          # transpose slots per PSUM tile (1 bank fp16)

_cached_nc = None


def _build():
    import concourse.bass as bass
    import concourse.mybir as mybir
    import concourse.tile as tile
    from concourse import bacc
    from concourse.masks import make_identity
    from concourse.tile import add_dep_helper

    fp32 = mybir.dt.float32
    fp16 = mybir.dt.float16
    mult = mybir.AluOpType.mult
    add = mybir.AluOpType.add

    nc = bacc.Bacc("TRN2", target_bir_lowering=False, debug=False)
    q = nc.dram_tensor("q", [BHC, N, D], fp32, kind="ExternalInput").ap()
    k = nc.dram_tensor("k", [BHC, N, D], fp32, kind="ExternalInput").ap()
    out = nc.dram_tensor("out", [BHC, NOUT, W, J], fp16, kind="ExternalOutput").ap()

    def raw(inst):
        return inst.ins if hasattr(inst, "ins") and not isinstance(inst.ins, list) else inst

    with tile.TileContext(nc) as tc:
        from contextlib import ExitStack

        with ExitStack() as ctx:
            singles = ctx.enter_context(tc.tile_pool(name="singles", bufs=1))
            kin_pool = ctx.enter_context(tc.tile_pool(name="kin", bufs=2))
            qin_pool = ctx.enter_context(tc.tile_pool(name="qin", bufs=2))
            kbf_pool = ctx.enter_context(tc.tile_pool(name="kbf", bufs=2))
            qbf_pool = ctx.enter_context(tc.tile_pool(name="qbf", bufs=2))
            kt_pool = ctx.enter_context(tc.tile_pool(name="kt", bufs=2))
            qt_pool = ctx.enter_context(tc.tile_pool(name="qt", bufs=2))
            sa_pool = ctx.enter_context(tc.tile_pool(name="stageA", bufs=3))
            sb_pool = ctx.enter_context(tc.tile_pool(name="stageB", bufs=2))
            sums_pool = ctx.enter_context(tc.tile_pool(name="sums", bufs=2))
            tpsum = ctx.enter_context(tc.tile_pool(name="tpsum", bufs=1, space="PSUM"))
            spsum = ctx.enter_context(tc.tile_pool(name="spsum", bufs=2, space="PSUM"))
            scrapp = ctx.enter_context(tc.tile_pool(name="scrap", bufs=1, space="PSUM"))

            identh = singles.tile([128, 128], fp16)
            make_identity(nc, identh)
            scrap = scrapp.tile([2, 2], fp32, tag="scrap")
            # absorb the gpsimd (ident) wait into PE's clock once
            nc.tensor.matmul(scrap, identh[:, :2], identh[:, :2], start=True, stop=True)

            def absorber(lhs2, rhs2, dep=None, why="absorber"):
                """1-wait PE matmul absorbing a cross-engine dependency."""
                mm = nc.tensor.matmul(scrap, lhs2, rhs2, start=True, stop=True)
                if dep is not None:
                    add_dep_helper(raw(mm), raw(dep), False, why)
                return mm

            group_rr = 0  # group counter for the normalize engine rotation
            for bh in range(BHC):
                # ---- contiguous loads: partition p <- tokens [64p, 64p+64) ----
                ktile = kin_pool.tile([128, 64, D], fp32, tag="kin")
                qtile = qin_pool.tile([128, 64, D], fp32, tag="qin")
                nc.sync.dma_start(out=ktile, in_=k[bh].rearrange("(p a) d -> p a d", p=128))
                nc.sync.dma_start(out=qtile, in_=q[bh].rearrange("(p a) d -> p a d", p=128))

                kbf = kbf_pool.tile([128, 64, D], fp16, tag="kbf")
                qbf = qbf_pool.tile([128, 64, D], fp16, tag="qbf")
                kt = kt_pool.tile([D, 64, 128], fp16, tag="kt")
                qt = qt_pool.tile([D, N], fp16, tag="qt")
                qt_pa = qt.rearrange("d (p a) -> d p a", p=128)

                # casts chunked in halves; k/q transpose tiles interleaved so
                # the ACT (k) and DVE (q) copy streams both start early
                for h in range(2):
                    hs = slice(32 * h, 32 * h + 32)
                    nc.gpsimd.tensor_copy(out=kbf[:, hs], in_=ktile[:, hs])
                    nc.gpsimd.tensor_copy(out=qbf[:, hs], in_=qtile[:, hs])
                    ab_k = absorber(kbf[:, 32 * h, :2], identh[:, :2], dep=None)
                    ab_q = absorber(qbf[:, 32 * h, :2], identh[:, :2], dep=None)
                    first_k = first_q = True
                    for a0 in range(32 * h, 32 * h + 32, TA):
                        # K tile: a-major contiguous both sides -> DVE 2x copy
                        tpk = tpsum.tile([D, TA, 128], fp16, tag="t")
                        for t in range(TA):
                            mm = nc.tensor.transpose(tpk[:, t, :], kbf[:, a0 + t, :], identh)
                            if first_k:
                                add_dep_helper(raw(mm), raw(ab_k), False, "k after absorber")
                                first_k = False
                        nc.vector.tensor_copy(out=kt[:, a0 : a0 + TA, :], in_=tpk)
                        # Q tile: token-major dst is strided -> ACT copy
                        tpq = tpsum.tile([D, TA, 128], fp16, tag="t2")
                        for t in range(TA):
                            mm = nc.tensor.transpose(tpq[:, t, :], qbf[:, a0 + t, :], identh)
                            if first_q:
                                add_dep_helper(raw(mm), raw(ab_q), False, "q after absorber")
                                first_q = False
                        nc.scalar.copy(
                            out=qt_pa[:, :, a0 : a0 + TA],
                            in_=tpq.rearrange("d t p -> d p t"),
                        )

                # absorbers soaking the DVE (qt) and ACT (kt) copy ticks so the
                # score matmuls' only cross-engine wait is the ACT psum-recycle
                absorber(kt[:, 0, :2], identh[:64, :2], dep=None)
                absorber(qt[:64, :2], identh[:64, :2], dep=None)

                # ---- per output-window group ----
                o0 = 0
                while o0 < NOUT:
                    gs = min(GS, NOUT - o0)
                    stage_a = sa_pool.tile([128, GS, J], fp16, tag="sa")
                    stage_b = sb_pool.tile([128, GS, J], fp16, tag="sb")
                    sums = sums_pool.tile([128, GS], fp32, tag="sums")
                    recip = sums_pool.tile([128, GS], fp32, tag="recip")
                    for p0 in range(0, gs, 2):
                        sc = spsum.tile([128, 2, 512], fp32, tag="s")
                        plens = []
                        for s2 in range(2):
                            s = p0 + s2
                            wi = REMAINING[o0 + s]
                            lo = max(0, 2 * wi - 2)
                            hi = min(128, 2 * wi + 4)
                            cols = 64 * (hi - lo)
                            plens.append(cols)
                            nc.tensor.matmul(
                                sc[:, s2, :cols],
                                qt[:, wi * W : (wi + 1) * W],
                                kt[:, :, lo:hi],
                                start=True,
                                stop=True,
                            )
                        # exp on ACT straight out of PSUM into the fp16 stage
                        if plens[0] == plens[1] == J:
                            nc.scalar.activation(
                                stage_a[:, p0 : p0 + 2, :],
                                sc[:, :, :J],
                                mybir.ActivationFunctionType.Exp,
                                scale=SCALE,
                            )
                        else:
                            for s2 in range(2):
                                nc.scalar.activation(
                                    stage_a[:, p0 + s2, : plens[s2]],
                                    sc[:, s2, : plens[s2]],
                                    mybir.ActivationFunctionType.Exp,
                                    scale=SCALE,
                                )
                                if plens[s2] < J:
                                    # zero the tail so the fold sums stay exact
                                    nc.vector.memset(stage_a[:, p0 + s2, plens[s2] :], 0.0)
                        # per-window flat L1 fold (2x packed): B[s,0:192] = A+A
                        for s2 in range(2):
                            s = p0 + s2
                            nc.vector.tensor_tensor(
                                out=stage_b[:, s, 0:192], in0=stage_a[:, s, 0:192],
                                in1=stage_a[:, s, 192:384], op=add)

                    # segmented tail reduce (1x) + reciprocal
                    nc.vector.tensor_reduce(
                        out=sums[:, :gs], in_=stage_b[:, :gs, 0:192],
                        axis=mybir.AxisListType.X, op=add)
                    nc.vector.reciprocal(recip[:, :gs], sums[:, :gs])

                    # ---- normalize A -> B, spread over DVE / ACT / Pool ----
                    for s in range(gs):
                        if s == 0 and group_rr % 2 == 0:
                            nc.scalar.mul(
                                stage_b[:, s, :], stage_a[:, s, :], recip[:, s : s + 1])
                        else:
                            nc.vector.tensor_scalar(
                                out=stage_b[:, s, :], in0=stage_a[:, s, :],
                                scalar1=recip[:, s : s + 1], scalar2=None, op0=mult)
                    group_rr += 1
                    dst = out[bh, o0 : o0 + gs].rearrange("w c j -> c w j")
                    nc.sync.dma_start(out=dst, in_=stage_b[:, :gs, :])
                    o0 += gs
    nc.compile()
    return nc


# ---- host-side permutation maps -------------------------------------------
# Output rows are already in query order.  Stage col a*6+dp holds key token
# 64*(2(w-1)+dp)+a, i.e. j_ref = 64*dp+a -> col(j) = (j%64)*6 + j//64.
# Window 0 (4 p-slots, j_ref>=128): col = ((j-128)%64)*4 + (j-128)//64.
# Window 63 (4 p-slots, j_ref<256): col = (j%64)*4 + j//64.
_JM = ((np.arange(J) % 64) * 6 + np.arange(J) // 64).astype(np.intp)
_J0 = (((np.arange(128, J) - 128) % 64) * 4 + (np.arange(128, J) - 128) // 64).astype(np.intp)
_J63 = ((np.arange(256) % 64) * 4 + np.arange(256) // 64).astype(np.intp)


def _assemble(raw):
    """raw: [BH, NOUT, 128, 384] fp16 device layout -> fp32 reference layout."""
    res = np.empty((BH, NOUT, W, J), np.float32)
    res[:, 1 : NOUT - 1] = raw[:, 1 : NOUT - 1][..., _JM]
    res[:, 0, :, :128] = 0.0
    res[:, 0, :, 128:] = raw[:, 0][..., _J0]
    res[:, NOUT - 1, :, :256] = raw[:, NOUT - 1][..., _J63]
    res[:, NOUT - 1, :, 256:] = 0.0
    return res


def _run(q, k, trace=False):
    from concourse.bass_utils import run_bass_kernel_spmd

    global _cached_nc
    if _cached_nc is None:
        _cached_nc = _build()
    nc = _cached_nc

    q = np.ascontiguousarray(np.asarray(q), dtype=np.float32).reshape(BH, N, D)
    k = np.ascontiguousarray(np.asarray(k), dtype=np.float32).reshape(BH, N, D)
    in_maps = [
        {
            "q": np.ascontiguousarray(q[c * BHC : (c + 1) * BHC]),
            "k": np.ascontiguousarray(k[c * BHC : (c + 1) * BHC]),
        }
        for c in range(NCORES)
    ]
    res = run_bass_kernel_spmd(nc, in_maps, core_ids=list(range(NCORES)), trace=trace)
    raw = np.concatenate([np.asarray(res.results[c]["out"]) for c in range(NCORES)], axis=0)
    return _assemble(raw), res


def kernel(q, k):
    out, _ = _run(q, k, trace=False)
    return out
